# revision 1
# baseline (speedup 1.0000x reference)
"""Trainium2 Bass kernel for nn_MemoryGraph (gnn_message_passing).

Sharding: neurons split across 8 cores (1024/core). Per step:
  - gather neighbor msgs rows from a replicated DRAM msgs buffer (dma_gather)
  - weighted-sum over K=32 neighbors on DVE/GPSIMD
  - 3 MLPs on PE (matmul1 feature-major -> hidden^T; matmul2 rows-layout)
  - h/ident/w_conn carries stay in SBUF; msgs AllGather'd to DRAM each step
Host side: layout prep (transposes, gather indices, inject precompute) in
numpy; output reassembly at the end.
"""

import numpy as np
from itertools import product

import concourse.bass as bass
import concourse.bacc as bacc
from concourse import mybir, tile, masks, library_config
from concourse.bass_utils import run_bass_kernel_spmd

# problem constants (hardcoded per harness contract)
N, K, D, D_ID = 8192, 32, 64, 32
H = 256
BS, T = 4, 8
NCORES = 8
NS = N // NCORES          # 1024 neurons per core
R = BS * NS               # 4096 rows per core (b-major: r = b*NS + n)
NCHUNK = R // 128         # 32 row-chunks of 128
NGRP = 16                 # gather groups per step: (b, jj) jj in 0..3
GIDX = 8192               # indices per gather group (2 blocks x 32 k x 128 n)

F32 = mybir.dt.float32
I16 = mybir.dt.int16
AF = mybir.ActivationFunctionType
ALU = mybir.AluOpType

_PROGRAM_CACHE = {}


def _build_program():
    nc = bacc.Bacc(
        "TRN2", target_bir_lowering=False, debug=False,
        num_devices=NCORES,
    )

    # ---------------- I/O declarations ----------------
    din = {}
    def dram_in(name, shape, dtype=F32):
        din[name] = nc.dram_tensor(name, shape, dtype, kind="ExternalInput")
        return din[name]

    h0T = dram_in("h0T", [D, R])
    h0R = dram_in("h0R", [R, D])
    msh0 = dram_in("msh0", [R, D])
    w0 = dram_in("w0", [128, NCHUNK, K])
    hebbT = dram_in("hebbT", [D_ID, R])
    identT_in = dram_in("identT", [D_ID, NS])
    injT = dram_in("injT", [T, D, R])
    idx_in = dram_in("idx", [128, NGRP, GIDX // 16], I16)
    dw1 = dram_in("dw1", [128, 2, H])
    dw2 = dram_in("dw2", [128, 2, K + 1 + D_ID])
    db1 = dram_in("db1", [128, 2])
    db2 = dram_in("db2", [1, 4, K + 1 + D_ID])
    sw1a = dram_in("sw1a", [128, H])
    sw1b = dram_in("sw1b", [96, H])
    sw2 = dram_in("sw2", [128, 2, D])
    sb1 = dram_in("sb1", [128, 2])
    sb2 = dram_in("sb2", [1, 4, D])
    mw1 = dram_in("mw1", [96, H])
    mw2 = dram_in("mw2", [128, 2, D])
    mb1 = dram_in("mb1", [128, 2])
    mb2 = dram_in("mb2", [1, 4, D])

    out_d = nc.dram_tensor("out", [T, R, D], F32, kind="ExternalOutput")

    rg = [list(range(NCORES))]
    MOD_O = K + 1 + D_ID  # 65

    with tile.TileContext(nc) as tc:
        with (
            tc.tile_pool(name="persist", bufs=1) as pp,
            tc.tile_pool(name="dram", bufs=1, space="DRAM") as dp,
            tc.tile_pool(name="gpool", bufs=3) as gp,
            tc.tile_pool(name="ppool", bufs=2) as ppl,
            tc.tile_pool(name="hid", bufs=2) as hp,
            tc.tile_pool(name="idxp", bufs=3) as ixp,
            tc.tile_pool(name="ps1", bufs=2, space="PSUM") as ps1p,
            tc.tile_pool(name="ps2", bufs=4, space="PSUM") as ps2p,
            tc.tile_pool(name="pst", bufs=2, space="PSUM") as pstp,
        ):
            # internal DRAM: per-batch msgs shard + all-gathered msgs
            mshb = [dp.tile([NS, D], F32, name=f"msh{b}", tag=f"msh{b}")
                    for b in range(BS)]
            mfullb = [dp.tile([N, D], F32, name=f"mfull{b}", tag=f"mfull{b}")
                      for b in range(BS)]
            # persistent tiles
            B = pp.tile([128, R], F32)       # [received(64); inject(64)]
            C = pp.tile([128, R], F32)       # [h(64); ide2(32); hebb(32)]
            wsig = pp.tile([128, NCHUNK, K], F32)
            identM = pp.tile([D_ID, NS], F32)
            hrows = pp.tile([128, NCHUNK, D], F32)
            omT = pp.tile([128, NCHUNK], F32)
            mrows = pp.tile([128, NCHUNK, D_ID], F32)
            msum = pp.tile([128, 8, D_ID], F32)
            ident128 = pp.tile([128, 128], F32)
            onesK = pp.tile([1, 128], F32)
            # weights
            t_dw1 = pp.tile([128, 2, H], F32)
            t_dw2 = pp.tile([128, 2, MOD_O], F32)
            t_db1 = pp.tile([128, 2], F32)
            t_db2 = pp.tile([1, 4, MOD_O], F32)
            t_sw1a = pp.tile([128, H], F32)
            t_sw1b = pp.tile([96, H], F32)
            t_sw2 = pp.tile([128, 2, D], F32)
            t_sb1 = pp.tile([128, 2], F32)
            t_sb2 = pp.tile([1, 4, D], F32)
            t_mw1 = pp.tile([96, H], F32)
            t_mw2 = pp.tile([128, 2, D], F32)
            t_mb1 = pp.tile([128, 2], F32)
            t_mb2 = pp.tile([1, 4, D], F32)

            # ---------------- preamble ----------------
            nc.gpsimd.load_library(library_config.mlp)
            masks.make_identity(nc, ident128[:])
            nc.vector.memset(onesK[:], 1.0)

            for tname, ttile in [
                ("dw1", t_dw1), ("dw2", t_dw2), ("db1", t_db1), ("db2", t_db2),
                ("sw1a", t_sw1a), ("sw1b", t_sw1b), ("sw2", t_sw2),
                ("sb1", t_sb1), ("sb2", t_sb2),
                ("mw1", t_mw1), ("mw2", t_mw2), ("mb1", t_mb1), ("mb2", t_mb2),
            ]:
                nc.sync.dma_start(out=ttile[:], in_=din[tname][:])

            nc.sync.dma_start(out=C[96:128, :], in_=hebbT[:])
            nc.sync.dma_start(out=C[0:D, :], in_=h0T[:])
            nc.sync.dma_start(out=identM[:], in_=identT_in[:])
            nc.sync.dma_start(out=wsig[:], in_=w0[:])
            nc.scalar.activation(out=wsig[:], in_=wsig[:], func=AF.Sigmoid)
            nc.sync.dma_start(
                out=hrows[:],
                in_=h0R[:].rearrange("(c p) d -> p c d", p=128),
            )
            # broadcast ident along batch into C feature rows
            ide_b = identM[:].unsqueeze(1).broadcast_to([D_ID, BS, NS])
            nc.scalar.copy(
                out=C[D:96, :].rearrange("p (b n) -> p b n", b=BS),
                in_=ide_b,
            )
            # initial msgs shards -> allgather (per batch)
            for b in range(BS):
                nc.sync.dma_start(out=mshb[b][:],
                                  in_=msh0[NS * b:NS * (b + 1), :])
                nc.gpsimd.collective_compute(
                    "AllGather", ALU.bypass, ins=[mshb[b].opt()],
                    outs=[mfullb[b].opt()], replica_groups=rg,
                )

            # ---------------- time loop ----------------
            for t in range(T):
                # inject slice for this step straight into B rows 64:128
                nc.sync.dma_start(out=B[D:2 * D, :], in_=injT[t])

                # ---- received: gather + weighted k-reduction ----
                for g, (b, jj) in enumerate(product(range(BS), range(4))):
                    eng = nc.gpsimd if g >= 10 else nc.vector
                    idxt = ixp.tile([128, GIDX // 16], I16, tag="idx")
                    nc.sync.dma_start(out=idxt[:], in_=idx_in[:, g, :])
                    G = gp.tile([128, GIDX // 128, D], F32, tag="G")
                    # HW cap: ~1024 indices per dma_gather instruction
                    for s in range(GIDX // 1024):
                        nc.gpsimd.dma_gather(
                            out_ap=G[:, 8 * s:8 * (s + 1), :],
                            in_ap=mfullb[b][:],
                            idxs_ap=idxt[:, 64 * s:64 * (s + 1)],
                            num_idxs=1024,
                            num_idxs_reg=1024,
                            elem_size=D,
                        )
                    c0 = b * 8 + jj * 2
                    P4 = G[:].rearrange("p (blk k) d -> p blk k d", blk=2)
                    wb = wsig[:, c0:c0 + 2, :].unsqueeze(3).broadcast_to(
                        [128, 2, K, D])
                    eng.tensor_tensor(out=P4, in0=P4, in1=wb, op=ALU.mult)
                    if eng is nc.vector:
                        # single strided reduce over k (X-axis, DVE only)
                        rcv = ppl.tile([128, 2, D], F32, tag="rcv")
                        nc.vector.tensor_reduce(
                            out=rcv[:],
                            in_=G[:].rearrange("p (blk k) d -> p blk d k",
                                               blk=2),
                            axis=mybir.AxisListType.X, op=ALU.add)
                        rsrc = [rcv[:, 0, :], rcv[:, 1, :]]
                    else:
                        kk = K
                        while kk > 1:
                            h = kk // 2
                            eng.tensor_tensor(
                                out=P4[:, :, 0:h, :], in0=P4[:, :, 0:h, :],
                                in1=P4[:, :, h:kk, :], op=ALU.add)
                            kk = h
                        rsrc = [P4[:, 0, 0, :], P4[:, 1, 0, :]]
                    # transpose received rows -> feature-major into B
                    tr = pstp.tile([64, 256], F32, tag="tr")
                    for blk in range(2):
                        nc.tensor.transpose(
                            tr[:, 128 * blk:128 * (blk + 1)],
                            rsrc[blk],
                            ident128[:],
                        )
                    nc.scalar.copy(
                        out=B[0:D, 256 * g:256 * (g + 1)], in_=tr[:])

                # ---- mod MLP: matmul1 (feature-major) ----
                modH = hp.tile([128, 2, R], F32, tag="hid")
                for m in range(2):
                    for ni in range(8):
                        ps = ps1p.tile([128, 512], F32, tag="mm1")
                        sl = slice(512 * ni, 512 * (ni + 1))
                        nc.tensor.matmul(
                            ps[:], t_dw1[:, 0, 128 * m:128 * (m + 1)],
                            C[:, sl], start=True, stop=False)
                        nc.tensor.matmul(
                            ps[:], t_dw1[:, 1, 128 * m:128 * (m + 1)],
                            B[:, sl], start=False, stop=True)
                        nc.scalar.activation(
                            out=modH[:, m, sl], in_=ps[:], func=AF.Silu,
                            bias=t_db1[:, m:m + 1])

                # ---- state MLP matmul1 (needs B received + C old-h/old... )
                # NOTE: C still holds old h + old ide2 here; ide2 update below
                # must come first per reference (state uses NEW ident).
                # So: mod matmul2 + ident update BEFORE state matmul1.

                # ---- mod matmul2: 4 chunks per psum bank (128-f32 stride
                # per chunk keeps PE psum writes 512B-aligned) ----
                for q in range(8):
                    ps2 = ps2p.tile([128, 4, 128], F32, tag="mm2")
                    for i in range(4):
                        csl = slice(128 * (4 * q + i), 128 * (4 * q + i + 1))
                        nc.tensor.matmul(ps2[:, i, 0:MOD_O], modH[:, 0, csl],
                                         t_dw2[:, 0, :], start=True,
                                         stop=False, skip_group_check=True)
                        nc.tensor.matmul(ps2[:, i, 0:MOD_O], modH[:, 1, csl],
                                         t_dw2[:, 1, :], start=False,
                                         stop=False, skip_group_check=True)
                        nc.tensor.matmul(ps2[:, i, 0:MOD_O], onesK[:],
                                         t_db2[:, 0, :], start=False,
                                         stop=True, skip_group_check=True)
                    qsl = slice(4 * q, 4 * (q + 1))
                    nc.scalar.activation(out=wsig[:, qsl, :],
                                         in_=ps2[:, :, 0:K], func=AF.Sigmoid)
                    nc.scalar.activation(out=omT[:, qsl], in_=ps2[:, :, K],
                                         func=AF.Sigmoid, scale=-1.0)
                    nc.vector.tensor_copy(out=mrows[:, qsl, :],
                                           in_=ps2[:, :, K + 1:MOD_O])

                # ---- ident update: mean over batch, transpose, accumulate
                mv = mrows[:].rearrange("p (b j) f -> p b j f", b=BS)
                nc.gpsimd.tensor_tensor(out=msum[:], in0=mv[:, 0],
                                        in1=mv[:, 1], op=ALU.add)
                nc.gpsimd.tensor_tensor(out=msum[:], in0=msum[:],
                                        in1=mv[:, 2], op=ALU.add)
                nc.gpsimd.tensor_tensor(out=msum[:], in0=msum[:],
                                        in1=mv[:, 3], op=ALU.add)
                for j in range(8):
                    it = pstp.tile([D_ID, 128], F32, tag="tr")
                    nc.tensor.transpose(it[:], msum[:, j, :], ident128[:])
                    nc.vector.scalar_tensor_tensor(
                        out=identM[:, 128 * j:128 * (j + 1)],
                        in0=it[:], scalar=1.0 / BS,
                        in1=identM[:, 128 * j:128 * (j + 1)],
                        op0=ALU.mult, op1=ALU.add)
                # new ident -> C rows 64:96 (state/msg this step; mod next)
                ide_b2 = identM[:].unsqueeze(1).broadcast_to([D_ID, BS, NS])
                nc.scalar.copy(
                    out=C[D:96, :].rearrange("p (b n) -> p b n", b=BS),
                    in_=ide_b2)

                # ---- state MLP matmul1 ----
                stateH = hp.tile([128, 2, R], F32, tag="hid")
                for m in range(2):
                    for ni in range(8):
                        ps = ps1p.tile([128, 512], F32, tag="mm1")
                        sl = slice(512 * ni, 512 * (ni + 1))
                        nc.tensor.matmul(
                            ps[:], t_sw1a[:, 128 * m:128 * (m + 1)],
                            B[:, sl], start=True, stop=False)
                        nc.tensor.matmul(
                            ps[:], t_sw1b[:, 128 * m:128 * (m + 1)],
                            C[0:96, sl], start=False, stop=True)
                        nc.scalar.activation(
                            out=stateH[:, m, sl], in_=ps[:], func=AF.Silu,
                            bias=t_sb1[:, m:m + 1])

                # ---- state matmul2 (rows) + tanh ----
                tanhR = ppl.tile([128, NCHUNK, D], F32, tag="rowsD")
                for q in range(8):
                    ps3 = ps2p.tile([128, 4, D], F32, tag="mm2")
                    for i in range(4):
                        csl = slice(128 * (4 * q + i), 128 * (4 * q + i + 1))
                        nc.tensor.matmul(ps3[:, i, :], stateH[:, 0, csl],
                                         t_sw2[:, 0, :], start=True,
                                         stop=False, skip_group_check=True)
                        nc.tensor.matmul(ps3[:, i, :], stateH[:, 1, csl],
                                         t_sw2[:, 1, :], start=False,
                                         stop=False, skip_group_check=True)
                        nc.tensor.matmul(ps3[:, i, :], onesK[:],
                                         t_sb2[:, 0, :], start=False,
                                         stop=True, skip_group_check=True)
                    nc.scalar.activation(out=tanhR[:, 4 * q:4 * (q + 1), :],
                                         in_=ps3[:], func=AF.Tanh)

                # ---- h_new = h + om*(tanh - h) (rows layout) ----
                omb = omT[:].unsqueeze(2).broadcast_to([128, NCHUNK, D])
                nc.gpsimd.tensor_tensor(out=tanhR[:], in0=tanhR[:],
                                        in1=hrows[:], op=ALU.subtract)
                nc.gpsimd.tensor_tensor(out=tanhR[:], in0=tanhR[:], in1=omb,
                                        op=ALU.mult)
                nc.gpsimd.tensor_tensor(out=hrows[:], in0=hrows[:],
                                        in1=tanhR[:], op=ALU.add)

                # output word_states for this step
                nc.sync.dma_start(
                    out=out_d[t].rearrange("(c p) d -> p c d", p=128),
                    in_=hrows[:])

                # h_new^T -> C rows 0:64, then copy into A rows 32:96
                for q in range(8):
                    ht = pstp.tile([64, 512], F32, tag="tr")
                    for i in range(4):
                        nc.tensor.transpose(
                            ht[:, 128 * i:128 * (i + 1)],
                            hrows[:, 4 * q + i, :], ident128[:])
                    nc.scalar.copy(out=C[0:D, 512 * q:512 * (q + 1)],
                                   in_=ht[:])

                # ---- msg MLP ----
                msgsR = ppl.tile([128, NCHUNK, D], F32, tag="rowsD")
                msgH = hp.tile([128, 2, R], F32, tag="hid")
                for m in range(2):
                    for ni in range(8):
                        ps = ps1p.tile([128, 512], F32, tag="mm1")
                        sl = slice(512 * ni, 512 * (ni + 1))
                        nc.tensor.matmul(
                            ps[:], t_mw1[:, 128 * m:128 * (m + 1)],
                            C[0:96, sl], start=True, stop=True)
                        nc.scalar.activation(
                            out=msgH[:, m, sl], in_=ps[:], func=AF.Silu,
                            bias=t_mb1[:, m:m + 1])
                for q in range(8):
                    ps4 = ps2p.tile([128, 4, D], F32, tag="mm2")
                    for i in range(4):
                        csl = slice(128 * (4 * q + i), 128 * (4 * q + i + 1))
                        nc.tensor.matmul(ps4[:, i, :], msgH[:, 0, csl],
                                         t_mw2[:, 0, :], start=True,
                                         stop=False, skip_group_check=True)
                        nc.tensor.matmul(ps4[:, i, :], msgH[:, 1, csl],
                                         t_mw2[:, 1, :], start=False,
                                         stop=False, skip_group_check=True)
                        nc.tensor.matmul(ps4[:, i, :], onesK[:],
                                         t_mb2[:, 0, :], start=False,
                                         stop=True, skip_group_check=True)
                    nc.scalar.activation(out=msgsR[:, 4 * q:4 * (q + 1), :],
                                         in_=ps4[:], func=AF.Tanh)
                    if t < T - 1 and q % 2 == 1:
                        b = q // 2
                        nc.sync.dma_start(
                            out=mshb[b][:].rearrange("(j p) d -> p j d",
                                                     p=128),
                            in_=msgsR[:, 8 * b:8 * (b + 1), :])
                        nc.gpsimd.collective_compute(
                            "AllGather", ALU.bypass, ins=[mshb[b].opt()],
                            outs=[mfullb[b].opt()], replica_groups=rg,
                        )

    nc.finalize()
    return nc


def _dw1p(dw1):
    # C feature order is [h(0:64); ide(64:96); hebb(96:128)]; dw1's input
    # rows are [hebb(0:32); h(32:96); ide(96:128); received; inject].
    return np.concatenate([dw1[32:96], dw1[96:128], dw1[0:32], dw1[128:]],
                          axis=0)


def _prep_inputs(inputs):
    """Build the per-core input maps from the full problem inputs."""
    cc = np.asarray(inputs["cc_signals"], dtype=np.float32)
    h0 = np.asarray(inputs["h0"], dtype=np.float32)
    msgs0 = np.asarray(inputs["msgs0"], dtype=np.float32)
    w_conn0 = np.asarray(inputs["w_conn0"], dtype=np.float32)
    hebb = np.asarray(inputs["hebbian"], dtype=np.float32)
    ident = np.asarray(inputs["identity"], dtype=np.float32)
    conn = np.asarray(inputs["conn_indices"]).astype(np.int64)

    def f32(x):
        return np.ascontiguousarray(x, dtype=np.float32)

    shared = {
        "dw1": f32(_dw1p(np.asarray(inputs["dw1"])).reshape(2, 128, H)
                   .transpose(1, 0, 2)),
        "dw2": f32(np.asarray(inputs["dw2"]).reshape(2, 128, K + 1 + D_ID)
                   .transpose(1, 0, 2)),
        "db1": f32(np.asarray(inputs["db1"]).reshape(2, 128).T),
        "db2": f32(np.tile(np.asarray(inputs["db2"]).reshape(1, 1, K + 1 + D_ID), (1, 4, 1))),
        "sw1a": f32(np.asarray(inputs["sw1"])[:128]),
        "sw1b": f32(np.asarray(inputs["sw1"])[128:224]),
        "sw2": f32(np.asarray(inputs["sw2"]).reshape(2, 128, D)
                   .transpose(1, 0, 2)),
        "sb1": f32(np.asarray(inputs["sb1"]).reshape(2, 128).T),
        "sb2": f32(np.tile(np.asarray(inputs["sb2"]).reshape(1, 1, D), (1, 4, 1))),
        "mw1": f32(np.asarray(inputs["mw1"])),
        "mw2": f32(np.asarray(inputs["mw2"]).reshape(2, 128, D)
                   .transpose(1, 0, 2)),
        "mb1": f32(np.asarray(inputs["mb1"]).reshape(2, 128).T),
        "mb2": f32(np.tile(np.asarray(inputs["mb2"]).reshape(1, 1, D), (1, 4, 1))),
    }

    seg = cc.reshape(BS, T, N // 512, D)  # [b, t, slice, d]
    in_maps = []
    for c in range(NCORES):
        sh = slice(c * NS, (c + 1) * NS)
        h0s = h0[:, sh]                       # [4, 1024, 64]
        m = dict(shared)
        m["h0T"] = f32(h0s.transpose(2, 0, 1).reshape(D, R))
        m["h0R"] = f32(h0s.reshape(R, D))
        m["msh0"] = f32(msgs0[:, sh].reshape(R, D))
        m["w0"] = f32(w_conn0[:, sh].reshape(BS, 8, 128, K)
                      .transpose(2, 0, 1, 3).reshape(128, NCHUNK, K))
        m["hebbT"] = f32(hebb[:, sh].transpose(2, 0, 1).reshape(D_ID, R))
        m["identT"] = f32(ident[sh].T)

        injT = np.empty((T, D, BS, NS), dtype=np.float32)
        half0 = seg[:, :, 2 * c]              # [b, t, d]
        half1 = seg[:, :, 2 * c + 1]
        injT[:, :, :, :512] = half0.transpose(1, 2, 0)[:, :, :, None]
        injT[:, :, :, 512:] = half1.transpose(1, 2, 0)[:, :, :, None]
        m["injT"] = f32(injT.reshape(T, D, R))

        tgt = conn[sh]                        # [1024, 32] global neuron ids
        idx_all = np.empty((128, NGRP, GIDX // 16), dtype=np.int16)
        for g, (b, jj) in enumerate(product(range(BS), range(4))):
            rows_b = tgt[jj * 256:(jj + 1) * 256]  # per-b buffers: row == id
            lin = rows_b.reshape(2, 128, K).transpose(0, 2, 1).reshape(GIDX)
            wrapped = lin.reshape(GIDX // 16, 16).T.astype(np.int16)
            idx_all[:, g, :] = np.tile(wrapped, (8, 1))
        m["idx"] = idx_all
        in_maps.append(m)
    return in_maps


def kernel(**inputs) -> np.ndarray:
    key = "prog"
    if key not in _PROGRAM_CACHE:
        _PROGRAM_CACHE[key] = _build_program()
    nc = _PROGRAM_CACHE[key]

    in_maps = _prep_inputs(inputs)
    res = run_bass_kernel_spmd(nc, in_maps, list(range(NCORES)))
    full = np.empty((BS, T, N, D), dtype=np.float32)
    for c in range(NCORES):
        o = np.asarray(res.results[c]["out"]).reshape(T, BS, NS, D)
        full[:, :, c * NS:(c + 1) * NS, :] = o.transpose(1, 0, 2, 3)
    return full.reshape(BS, T, N // 64, 64 * D)



# revision 9
# speedup vs baseline: 1.0669x; 1.0669x over previous
"""Trainium2 Bass kernel for nn_MemoryGraph (gnn_message_passing).

Sharding: neurons split across 8 cores (1024/core), all 4 batches local.
msgs stored batch-interleaved [n, b, d] in bf16 so ONE 512B gather
descriptor per edge (n,k) serves all 4 batches; one AllGather per step.

Per step (per core, R = 4*1024 rows, cols r = b*1024 + n, feature-major):
  - gather neighbor msg rows (bf16) from DRAM mfull [8192, 4*64]
  - received = per-target K-weighted sums as tiny PE matmuls
    (stationary = gathered [32k x 64d] block, moving = block-diag w column)
  - 3 MLPs on PE in float32r (1 cycle/row at >=256 free)
  - mod MLP2 emitted feature-major: w^T (wrapped for tiny-mm moving),
    1-decay replicated via replicated-column stationary, ident delta
  - h kept feature-major in C[0:64]; output DMA'd as [T, 64, R]
Host side: layout prep in numpy; output reassembly at the end.
"""

import numpy as np
import ml_dtypes

import concourse.bass as bass
import concourse.bacc as bacc
from concourse import mybir, tile, library_config
from concourse.bass_utils import run_bass_kernel_spmd

# problem constants (hardcoded per harness contract)
N, K, D, D_ID = 8192, 32, 64, 32
H = 256
BS, T = 4, 8
NCORES = 8
NS = N // NCORES          # 1024 neurons per core
R = BS * NS               # 4096 rows per core (b-major: r = b*NS + n)
NG = 32                   # gather instructions per step (1024 idx each)
EL = BS * D               # gathered elem: 256 bf16 = 512B (all batches)

F32 = mybir.dt.float32
F32R = mybir.dt.float32r
BF16 = mybir.dt.bfloat16
I16 = mybir.dt.int16
AF = mybir.ActivationFunctionType
ALU = mybir.AluOpType

_PROGRAM_CACHE = {}


def _build_program():
    nc = bacc.Bacc(
        "TRN2", target_bir_lowering=False, debug=False,
        num_devices=NCORES,
    )

    din = {}
    def dram_in(name, shape, dtype=F32):
        din[name] = nc.dram_tensor(name, shape, dtype, kind="ExternalInput")
        return din[name]

    h0T = dram_in("h0T", [D, R])
    hebbT = dram_in("hebbT", [D_ID, R])
    identT_in = dram_in("identT", [D_ID, NS])
    injT = dram_in("injT", [T, D, R])
    msh0 = dram_in("msh0", [NS, EL])
    w0blk = dram_in("w0blk", [128, 4, BS, NS // 4])
    idx_in = dram_in("idx", [128, 2048], I16)
    dw1C = dram_in("dw1C", [128, H])
    dw1B = dram_in("dw1B", [128, H])
    db1 = dram_in("db1", [128, 2])
    dw2wT = dram_in("dw2wT", [128, 2, K])
    db2wT = dram_in("db2wT", [128, 1])
    dw2om = dram_in("dw2om", [128, 2, D])
    db2om = dram_in("db2om", [D, 1])
    dw2de = dram_in("dw2de", [128, 2, D_ID])
    db2de = dram_in("db2de", [D_ID, 1])
    sw1B = dram_in("sw1B", [128, H])
    sw1C = dram_in("sw1C", [96, H])
    sb1 = dram_in("sb1", [128, 2])
    sw2 = dram_in("sw2", [128, 2, D])
    sb2 = dram_in("sb2", [D, 1])
    mw1 = dram_in("mw1", [96, H])
    mb1 = dram_in("mb1", [128, 2])
    mw2 = dram_in("mw2", [128, 2, D])
    mb2 = dram_in("mb2", [1, D])

    out_d = nc.dram_tensor("out", [T, D, R], F32, kind="ExternalOutput")

    rg = [list(range(NCORES))]

    with tile.TileContext(nc) as tc:
        with (
            tc.tile_pool(name="persist", bufs=1) as pp,
            tc.tile_pool(name="dram", bufs=1, space="DRAM") as dp,
            tc.tile_pool(name="gpool", bufs=2) as gp,
            tc.tile_pool(name="hid", bufs=2) as hp,
            tc.tile_pool(name="psA", bufs=2, space="PSUM") as psA,
            tc.tile_pool(name="psB", bufs=2, space="PSUM") as psB,
            tc.tile_pool(name="ps1", bufs=2, space="PSUM") as ps1p,
            tc.tile_pool(name="psW", bufs=1, space="PSUM") as psWp,
        ):
            mshard = dp.tile([NS, EL], F32, name="mshard", tag="mshard")
            mfull = dp.tile([N, EL], F32, name="mfull", tag="mfull")

            # persistent SBUF
            C = pp.tile([128, R], F32)        # [h(64); ide(32); hebb(32)]
            B = pp.tile([128, R], F32)        # [received(64); inject(64)]
            wTblk = pp.tile([128, 4, BS, NS // 4], F32)   # block-diag w^T
            identM = pp.tile([D_ID, NS], F32)
            omR = pp.tile([D, R], F32)        # (1-decay) replicated over d
            Ttanh = pp.tile([D, R], F32)
            dI = pp.tile([D_ID, 2, NS], F32)  # per-batch ident deltas
            msgs = pp.tile([128, NS // 128, EL], F32)
            idxT = pp.tile([128, 2048], I16)
            onesK = pp.tile([1, 128], F32)
            # weights
            t_dw1C = pp.tile([128, H], F32)
            t_dw1B = pp.tile([128, H], F32)
            t_db1 = pp.tile([128, 2], F32)
            t_dw2wT = pp.tile([128, 2, K], F32)
            t_db2wT = pp.tile([128, 1], F32)
            t_dw2om = pp.tile([128, 2, D], F32)
            t_db2om = pp.tile([D, 1], F32)
            t_dw2de = pp.tile([128, 2, D_ID], F32)
            t_db2de = pp.tile([D_ID, 1], F32)
            t_sw1B = pp.tile([128, H], F32)
            t_sw1C = pp.tile([96, H], F32)
            t_sb1 = pp.tile([128, 2], F32)
            t_sw2 = pp.tile([128, 2, D], F32)
            t_sb2 = pp.tile([D, 1], F32)
            t_mw1 = pp.tile([96, H], F32)
            t_mb1 = pp.tile([128, 2], F32)
            t_mw2 = pp.tile([128, 2, D], F32)
            t_mb2 = pp.tile([1, D], F32)

            # ---------------- preamble ----------------
            nc.gpsimd.load_library(library_config.mlp)
            nc.vector.memset(onesK[:], 1.0)

            for tname, ttile in [
                ("dw1C", t_dw1C), ("dw1B", t_dw1B), ("db1", t_db1),
                ("dw2wT", t_dw2wT), ("db2wT", t_db2wT),
                ("dw2om", t_dw2om), ("db2om", t_db2om),
                ("dw2de", t_dw2de), ("db2de", t_db2de),
                ("sw1B", t_sw1B), ("sw1C", t_sw1C), ("sb1", t_sb1),
                ("sw2", t_sw2), ("sb2", t_sb2),
                ("mw1", t_mw1), ("mb1", t_mb1),
                ("mw2", t_mw2), ("mb2", t_mb2),
            ]:
                nc.sync.dma_start(out=ttile[:], in_=din[tname][:])

            nc.sync.dma_start(out=C[0:D, :], in_=h0T[:])
            nc.sync.dma_start(out=C[96:128, :], in_=hebbT[:])
            nc.sync.dma_start(out=identM[:], in_=identT_in[:])
            nc.sync.dma_start(out=wTblk[:], in_=w0blk[:])
            nc.sync.dma_start(out=idxT[:], in_=idx_in[:])
            ide_b = identM[:].unsqueeze(1).broadcast_to([D_ID, BS, NS])
            nc.scalar.copy(
                out=C[D:96, :].rearrange("p (b n) -> p b n", b=BS),
                in_=ide_b)
            nc.sync.dma_start(out=mshard[:], in_=msh0[:])
            nc.gpsimd.collective_compute(
                "AllGather", ALU.bypass, ins=[mshard.opt()],
                outs=[mfull.opt()], replica_groups=rg)

            # ---------------- time loop ----------------
            for t in range(T):
                nc.sync.dma_start(out=B[D:2 * D, :], in_=injT[t])

                # ---- received: gather + tiny weighted matmuls ----
                for half in range(2):
                    pA = psA.tile([128, 512], F32, tag="psA")
                    pB = psB.tile([128, 512], F32, tag="psB")
                    for gi in range(16):
                        g = 16 * half + gi
                        G = gp.tile([128, 8, EL], F32, tag="G")
                        nc.gpsimd.dma_gather(
                            out_ap=G[:],
                            in_ap=mfull[:],
                            idxs_ap=idxT[:, 64 * g:64 * (g + 1)],
                            num_idxs=1024,
                            num_idxs_reg=1024,
                            elem_size=EL,
                        )
                        for cp in range(8):
                            col = 4 * (8 * gi + cp)
                            for b in range(BS):
                                ps = pA if b < 2 else pB
                                po = 64 * (b % 2)
                                c_glob = 8 * g + cp
                                nc.tensor.matmul(
                                    ps[po:po + 64, col:col + 4],
                                    G[:, cp, D * b:D * (b + 1)],
                                    wTblk[:, :, b, c_glob],
                                    start=True, stop=True,
                                    skip_group_check=True)
                    # drain received -> B rows 0:64 (b-major cols)
                    for b in range(BS):
                        ps = pA if b < 2 else pB
                        po = 64 * (b % 2)
                        nc.scalar.copy(
                            out=B[0:D, NS * b + 512 * half:
                                  NS * b + 512 * (half + 1)],
                            in_=ps[po:po + 64, :])
                    # mod MLP1 for this half's drained chunks
                    if half == 0:
                        modH = hp.tile([128, 2, R], F32, tag="hid")
                    for b in range(BS):
                        sl = slice(NS * b + 512 * half,
                                   NS * b + 512 * (half + 1))
                        for m in range(2):
                            ps = ps1p.tile([128, 512], F32, tag="mm1")
                            nc.tensor.matmul(
                                ps[:], t_dw1C[:, 128 * m:128 * (m + 1)],
                                C[:, sl], start=True, stop=False)
                            nc.tensor.matmul(
                                ps[:], t_dw1B[:, 128 * m:128 * (m + 1)],
                                B[:, sl], start=False, stop=True)
                            nc.scalar.activation(
                                out=modH[:, m, sl], in_=ps[:], func=AF.Silu,
                                bias=t_db1[:, m:m + 1])

                # ---- mod MLP2 (feature-major outputs) ----
                modHr = modH[:].rearrange("p m (b c g) -> p m b c g",
                                          b=BS, g=4)
                # w^T wrapped: psW[32g+k, b, c] = w[b, 4c+g, k]
                pW = psWp.tile([128, BS, NS // 4], F32, tag="wT")
                for g in range(4):
                    for b in range(BS):
                        for m in range(2):
                            nc.tensor.matmul(
                                pW[32 * g:32 * (g + 1), b, :],
                                t_dw2wT[:, m, :],
                                modHr[:, m, b, :, g],
                                start=(m == 0), stop=(m == 1),
                                skip_group_check=True,
                                tile_position=(0, 32 * g))
                # om = 1 - decay, replicated over 64 partitions via
                # column-replicated stationary
                for q in range(8):
                    sl = slice(512 * q, 512 * (q + 1))
                    ps = ps1p.tile([128, 512], F32, tag="mm1")
                    for m in range(2):
                        nc.tensor.matmul(
                            ps[0:D, :], t_dw2om[:, m, :],
                            modH[:, m, sl],
                            start=(m == 0), stop=(m == 1),
                            skip_group_check=True)
                    nc.scalar.activation(
                        out=omR[:, sl], in_=ps[0:D, :], func=AF.Sigmoid,
                        scale=-1.0, bias=t_db2om[:])
                # ident delta per batch (fm [32, n]); dI slot0 accumulates
                for b in range(BS):
                    slot = 0 if b == 0 else 1
                    for q in range(2):
                        sl = slice(NS * b + 512 * q, NS * b + 512 * (q + 1))
                        ps = ps1p.tile([128, 512], F32, tag="mm1")
                        for m in range(2):
                            nc.tensor.matmul(
                                ps[0:D_ID, :], t_dw2de[:, m, :],
                                modH[:, m, sl],
                                start=(m == 0), stop=(m == 1),
                                skip_group_check=True)
                        nc.scalar.activation(
                            out=dI[:, slot, 512 * q:512 * (q + 1)],
                            in_=ps[0:D_ID, :], func=AF.Identity,
                            bias=(t_db2de[:] if b == 0 else 0.0))
                    if b > 0:
                        nc.vector.tensor_tensor(
                            out=dI[:, 0, :], in0=dI[:, 0, :],
                            in1=dI[:, 1, :], op=ALU.add)
                # w^T sigmoid into block-diag moving tile (bf16)
                for g in range(4):
                    nc.scalar.activation(
                        out=wTblk[32 * g:32 * (g + 1), g, :, :],
                        in_=pW[32 * g:32 * (g + 1), :, :], func=AF.Sigmoid,
                        bias=t_db2wT[32 * g:32 * (g + 1), :])

                # ---- ident update + broadcast to C rows 64:96 ----
                nc.vector.scalar_tensor_tensor(
                    out=identM[:], in0=dI[:, 0, :], scalar=1.0 / BS,
                    in1=identM[:], op0=ALU.mult, op1=ALU.add)
                ide_b2 = identM[:].unsqueeze(1).broadcast_to([D_ID, BS, NS])
                nc.scalar.copy(
                    out=C[D:96, :].rearrange("p (b n) -> p b n", b=BS),
                    in_=ide_b2)

                # ---- state MLP1 ----
                stateH = hp.tile([128, 2, R], F32, tag="hid")
                for q in range(8):
                    sl = slice(512 * q, 512 * (q + 1))
                    for m in range(2):
                        ps = ps1p.tile([128, 512], F32, tag="mm1")
                        nc.tensor.matmul(
                            ps[:], t_sw1B[:, 128 * m:128 * (m + 1)],
                            B[:, sl], start=True, stop=False)
                        nc.tensor.matmul(
                            ps[:], t_sw1C[:, 128 * m:128 * (m + 1)],
                            C[0:96, sl], start=False, stop=True)
                        nc.scalar.activation(
                            out=stateH[:, m, sl], in_=ps[:], func=AF.Silu,
                            bias=t_sb1[:, m:m + 1])

                # ---- state MLP2 feature-major + tanh ----
                for q in range(8):
                    sl = slice(512 * q, 512 * (q + 1))
                    ps = ps1p.tile([128, 512], F32, tag="mm1")
                    for m in range(2):
                        nc.tensor.matmul(
                            ps[0:D, :], t_sw2[:, m, :],
                            stateH[:, m, sl],
                            start=(m == 0), stop=(m == 1),
                            skip_group_check=True)
                    nc.scalar.activation(
                        out=Ttanh[:, sl], in_=ps[0:D, :], func=AF.Tanh,
                        bias=t_sb2[:])

                # ---- h_new = h + om*(tanh - h), feature-major in C ----
                nc.gpsimd.tensor_tensor(out=Ttanh[:], in0=Ttanh[:],
                                        in1=C[0:D, :], op=ALU.subtract)
                nc.gpsimd.tensor_tensor(out=Ttanh[:], in0=Ttanh[:],
                                        in1=omR[:], op=ALU.mult)
                nc.vector.tensor_tensor(out=C[0:D, :], in0=C[0:D, :],
                                        in1=Ttanh[:], op=ALU.add)

                # output word_states for this step (feature-major)
                nc.sync.dma_start(out=out_d[t], in_=C[0:D, :])

                # ---- msg MLP ----
                msgH = hp.tile([128, 2, R], F32, tag="hid")
                for q in range(8):
                    sl = slice(512 * q, 512 * (q + 1))
                    for m in range(2):
                        ps = ps1p.tile([128, 512], F32, tag="mm1")
                        nc.tensor.matmul(
                            ps[:], t_mw1[:, 128 * m:128 * (m + 1)],
                            C[0:96, sl], start=True, stop=True)
                        nc.scalar.activation(
                            out=msgH[:, m, sl], in_=ps[:], func=AF.Silu,
                            bias=t_mb1[:, m:m + 1])
                # msg MLP2 rows, n-major interleaved-by-batch output
                for j in range(NS // 128):
                    ps = ps1p.tile([128, 512], F32, tag="mm1")
                    for b in range(BS):
                        rsl = slice(NS * b + 128 * j, NS * b + 128 * (j + 1))
                        nc.tensor.matmul(
                            ps[:, D * b:D * (b + 1)], msgH[:, 0, rsl],
                            t_mw2[:, 0, :], start=True, stop=False,
                            skip_group_check=True)
                        nc.tensor.matmul(
                            ps[:, D * b:D * (b + 1)], msgH[:, 1, rsl],
                            t_mw2[:, 1, :], start=False, stop=False,
                            skip_group_check=True)
                        nc.tensor.matmul(
                            ps[:, D * b:D * (b + 1)], onesK[:],
                            t_mb2[:], start=False, stop=True,
                            skip_group_check=True)
                    nc.scalar.activation(
                        out=msgs[:, j, :], in_=ps[:, 0:EL], func=AF.Tanh)

                # msgs -> DRAM shard -> AllGather
                nc.sync.dma_start(
                    out=mshard[:].rearrange("(j p) d -> p j d", p=128),
                    in_=msgs[:])
                nc.gpsimd.collective_compute(
                    "AllGather", ALU.bypass, ins=[mshard.opt()],
                    outs=[mfull.opt()], replica_groups=rg)

    nc.finalize()
    return nc


def _prep_inputs(inputs):
    """Build the per-core input maps from the full problem inputs."""
    cc = np.asarray(inputs["cc_signals"], dtype=np.float32)
    h0 = np.asarray(inputs["h0"], dtype=np.float32)
    msgs0 = np.asarray(inputs["msgs0"], dtype=np.float32)
    w_conn0 = np.asarray(inputs["w_conn0"], dtype=np.float32)
    hebb = np.asarray(inputs["hebbian"], dtype=np.float32)
    ident = np.asarray(inputs["identity"], dtype=np.float32)
    conn = np.asarray(inputs["conn_indices"]).astype(np.int64)

    def f32(x):
        return np.ascontiguousarray(x, dtype=np.float32)

    def bf16(x):
        return np.ascontiguousarray(
            np.asarray(x, dtype=np.float32).astype(ml_dtypes.bfloat16))

    dw1 = np.asarray(inputs["dw1"], dtype=np.float32)   # [256, 256]
    dw2 = np.asarray(inputs["dw2"], dtype=np.float32)   # [256, 65]
    db2 = np.asarray(inputs["db2"], dtype=np.float32)   # [65]
    sw1 = np.asarray(inputs["sw1"], dtype=np.float32)   # [224, 256]
    sw2 = np.asarray(inputs["sw2"], dtype=np.float32)   # [256, 64]
    mw1 = np.asarray(inputs["mw1"], dtype=np.float32)   # [96, 256]
    mw2 = np.asarray(inputs["mw2"], dtype=np.float32)   # [256, 64]

    # dw1 input order: [hebb(0:32), h(32:96), ide(96:128), rcv, inj]
    # C rows: [h, ide, hebb]; B rows: [rcv, inj]
    shared = {
        "dw1C": f32(np.concatenate([dw1[32:96], dw1[96:128], dw1[0:32]])),
        "dw1B": f32(dw1[128:256]),
        "db1": f32(np.asarray(inputs["db1"]).reshape(2, 128).T),
        "dw2wT": f32(dw2[:, 0:K].reshape(2, 128, K).transpose(1, 0, 2)),
        "db2wT": f32(np.tile(db2[0:K], 4).reshape(128, 1)),
        "dw2om": f32(np.repeat(dw2[:, K:K + 1], D, axis=1)
                      .reshape(2, 128, D).transpose(1, 0, 2)),
        "db2om": f32(np.full((D, 1), -db2[K])),
        "dw2de": f32(dw2[:, K + 1:].reshape(2, 128, D_ID).transpose(1, 0, 2)),
        "db2de": f32(db2[K + 1:].reshape(D_ID, 1)),
        "sw1B": f32(sw1[0:128]),
        "sw1C": f32(sw1[128:224]),
        "sb1": f32(np.asarray(inputs["sb1"]).reshape(2, 128).T),
        "sw2": f32(sw2.reshape(2, 128, D).transpose(1, 0, 2)),
        "sb2": f32(np.asarray(inputs["sb2"]).reshape(D, 1)),
        "mw1": f32(mw1),
        "mb1": f32(np.asarray(inputs["mb1"]).reshape(2, 128).T),
        "mw2": f32(mw2.reshape(2, 128, D).transpose(1, 0, 2)),
        "mb2": f32(np.asarray(inputs["mb2"]).reshape(1, D)),
    }

    def sigmoid(x):
        return 1.0 / (1.0 + np.exp(-x))

    seg = cc.reshape(BS, T, N // 512, D)  # [b, t, slice, d]
    in_maps = []
    for c in range(NCORES):
        sh = slice(c * NS, (c + 1) * NS)
        m = dict(shared)
        m["h0T"] = f32(h0[:, sh].transpose(2, 0, 1).reshape(D, R))
        m["hebbT"] = f32(hebb[:, sh].transpose(2, 0, 1).reshape(D_ID, R))
        m["identT"] = f32(ident[sh].T)

        injT = np.empty((T, D, BS, NS), dtype=np.float32)
        injT[:, :, :, :512] = seg[:, :, 2 * c].transpose(1, 2, 0)[:, :, :, None]
        injT[:, :, :, 512:] = seg[:, :, 2 * c + 1].transpose(1, 2, 0)[:, :, :, None]
        m["injT"] = f32(injT.reshape(T, D, R))

        # msgs0 interleaved [n, b, d]
        m["msh0"] = f32(msgs0[:, sh].transpose(1, 0, 2).reshape(NS, EL))

        # block-diag wrapped sigmoid(w0): blk[32g+k, g, b, c] = s(w0[b,4c+g,k])
        w0 = sigmoid(w_conn0[:, sh])            # [BS, NS, K]
        wr = w0.reshape(BS, NS // 4, 4, K)      # [b, c, g, k]
        blk = np.zeros((128, 4, BS, NS // 4), dtype=np.float32)
        for g in range(4):
            blk[32 * g:32 * (g + 1), g] = wr[:, :, g, :].transpose(2, 0, 1)
        m["w0blk"] = f32(blk)

        # gather indices: instr g covers targets 32g..32g+32;
        # lin[i] for i = 1024*g + 128*cp + 32*gp + k  -> conn[4*(8g+cp)+gp, k]
        tgt = conn[sh]                          # [1024, 32] global ids
        lin = tgt.reshape(NS // 4, 4, K).reshape(NG, 8, 4, K).reshape(-1)
        wrapped = lin.reshape(2048, 16).T.astype(np.int16)   # [16, 2048]
        m["idx"] = np.ascontiguousarray(np.tile(wrapped, (8, 1)))
        in_maps.append(m)
    return in_maps


def kernel(**inputs) -> np.ndarray:
    key = "prog"
    if key not in _PROGRAM_CACHE:
        _PROGRAM_CACHE[key] = _build_program()
    nc = _PROGRAM_CACHE[key]

    in_maps = _prep_inputs(inputs)
    res = run_bass_kernel_spmd(nc, in_maps, list(range(NCORES)))
    full = np.empty((BS, T, N, D), dtype=np.float32)
    for c in range(NCORES):
        o = np.asarray(res.results[c]["out"]).astype(np.float32)  # [T, D, R]
        o = o.reshape(T, D, BS, NS).transpose(2, 0, 3, 1)
        full[:, :, c * NS:(c + 1) * NS, :] = o
    return full.reshape(BS, T, N // 64, 64 * D)


# revision 13
# speedup vs baseline: 1.2449x; 1.1668x over previous
"""Trainium2 Bass kernel for nn_MemoryGraph (gnn_message_passing).

Sharding: neurons split across 8 cores (1024/core), all 4 batches local.
msgs stored batch-interleaved [n, b, d] in bf16 so ONE 512B gather
descriptor per edge (n,k) serves all 4 batches; one AllGather per step.

Per step (per core, R = 4*1024 rows, cols r = b*1024 + n, feature-major):
  - gather neighbor msg rows (bf16) from DRAM mfull [8192, 4*64]
  - received = per-target K-weighted sums as tiny PE matmuls
    (stationary = gathered [32k x 64d] block, moving = block-diag w column)
  - 3 MLPs on PE in float32r (1 cycle/row at >=256 free)
  - mod MLP2 emitted feature-major: w^T (wrapped for tiny-mm moving),
    1-decay replicated via replicated-column stationary, ident delta
  - h kept feature-major in C[0:64]; output DMA'd as [T, 64, R]
Host side: layout prep in numpy; output reassembly at the end.
"""

import numpy as np
import ml_dtypes

import concourse.bass as bass
import concourse.bacc as bacc
from concourse import mybir, tile, library_config
from concourse.bass_utils import run_bass_kernel_spmd

# problem constants (hardcoded per harness contract)
N, K, D, D_ID = 8192, 32, 64, 32
H = 256
BS, T = 4, 8
NCORES = 8
NS = N // NCORES          # 1024 neurons per core
R = BS * NS               # 4096 rows per core (b-major: r = b*NS + n)
NG = 32                   # gather instructions per step (1024 idx each)
EL = BS * D               # gathered elem: 256 bf16 = 512B (all batches)

F32 = mybir.dt.float32
F32R = mybir.dt.float32r
BF16 = mybir.dt.bfloat16
I16 = mybir.dt.int16
AF = mybir.ActivationFunctionType
ALU = mybir.AluOpType

_PROGRAM_CACHE = {}


def _build_program():
    nc = bacc.Bacc(
        "TRN2", target_bir_lowering=False, debug=False,
        num_devices=NCORES,
    )

    din = {}
    def dram_in(name, shape, dtype=F32):
        din[name] = nc.dram_tensor(name, shape, dtype, kind="ExternalInput")
        return din[name]

    h0T = dram_in("h0T", [D, R])
    hebbT = dram_in("hebbT", [D_ID, R], F32R)
    identT_in = dram_in("identT", [D_ID, NS])
    injT = dram_in("injT", [T, D, R], F32R)
    msh0 = dram_in("msh0", [NS, EL], F32R)
    w0blk = dram_in("w0blk", [128, 4, BS, NS // 4], F32R)
    idx_in = dram_in("idx", [128, 2048], I16)
    dw1C = dram_in("dw1C", [128, H], F32R)
    dw1B = dram_in("dw1B", [128, H], F32R)
    db1 = dram_in("db1", [128, 2])
    dw2wT = dram_in("dw2wT", [128, 2, K], F32R)
    db2wT = dram_in("db2wT", [K, 1])
    dw2om = dram_in("dw2om", [128, 2, D], F32R)
    db2om = dram_in("db2om", [D, 1])
    dw2de = dram_in("dw2de", [128, 2, D_ID], F32R)
    db2de = dram_in("db2de", [D_ID, 1])
    sw1B = dram_in("sw1B", [128, H], F32R)
    sw1C = dram_in("sw1C", [96, H], F32R)
    sb1 = dram_in("sb1", [128, 2])
    sw2 = dram_in("sw2", [128, 2, D], F32R)
    sb2 = dram_in("sb2", [D, 1])
    mw1 = dram_in("mw1", [96, H], F32R)
    mb1 = dram_in("mb1", [128, 2])
    mw2 = dram_in("mw2", [128, 2, D], F32R)
    mb2 = dram_in("mb2", [1, D], F32R)
    ones1 = dram_in("ones1", [1, 128], F32R)

    out_d = nc.dram_tensor("out", [T, D, R], F32, kind="ExternalOutput")

    rg = [list(range(NCORES))]

    with tile.TileContext(nc) as tc:
        with (
            tc.tile_pool(name="persist", bufs=1) as pp,
            tc.tile_pool(name="dram", bufs=1, space="DRAM") as dp,
            tc.tile_pool(name="gpool", bufs=2) as gp,
            tc.tile_pool(name="hid", bufs=1) as hp,
            tc.tile_pool(name="psR", bufs=4, space="PSUM") as psRp,
            tc.tile_pool(name="ps1", bufs=3, space="PSUM") as ps1p,
        ):
            mshard = dp.tile([NS, EL], F32R, name="mshard", tag="mshard")
            mfull = dp.tile([N, EL], F32R, name="mfull", tag="mfull")

            # persistent SBUF
            C = pp.tile([128, R], F32R)        # [h(64); ide(32); hebb(32)]
            B = pp.tile([128, R], F32R)        # [received(64); inject(64)]
            wTblk = pp.tile([128, 4, BS, NS // 4], F32R)   # block-diag w^T
            identM = pp.tile([D_ID, NS], F32)
            omR = pp.tile([D, R], F32)        # (1-decay) replicated over d
            Ttanh = pp.tile([D, R], F32)
            Hf = pp.tile([D, R], F32)         # fp32 master copy of h
            dI = pp.tile([D_ID, 2, NS], F32)  # per-batch ident deltas
            msgs = pp.tile([128, NS // 128, EL], F32R)
            idxT = pp.tile([128, 2048], I16)
            onesK = pp.tile([1, 128], F32R)
            onesK_src = onesK  # DMA'd from ones1 input
            # weights
            t_dw1C = pp.tile([128, H], F32R)
            t_dw1B = pp.tile([128, H], F32R)
            t_db1 = pp.tile([128, 2], F32)
            t_dw2wT = pp.tile([128, 2, K], F32R)
            t_db2wT = pp.tile([K, 1], F32)
            t_dw2om = pp.tile([128, 2, D], F32R)
            t_db2om = pp.tile([D, 1], F32)
            t_dw2de = pp.tile([128, 2, D_ID], F32R)
            t_db2de = pp.tile([D_ID, 1], F32)
            t_sw1B = pp.tile([128, H], F32R)
            t_sw1C = pp.tile([96, H], F32R)
            t_sb1 = pp.tile([128, 2], F32)
            t_sw2 = pp.tile([128, 2, D], F32R)
            t_sb2 = pp.tile([D, 1], F32)
            t_mw1 = pp.tile([96, H], F32R)
            t_mb1 = pp.tile([128, 2], F32)
            t_mw2 = pp.tile([128, 2, D], F32R)
            t_mb2 = pp.tile([1, D], F32R)

            # ---------------- preamble ----------------
            nc.gpsimd.load_library(library_config.mlp)

            for tname, ttile in [
                ("dw1C", t_dw1C), ("dw1B", t_dw1B), ("db1", t_db1),
                ("dw2wT", t_dw2wT), ("db2wT", t_db2wT),
                ("dw2om", t_dw2om), ("db2om", t_db2om),
                ("dw2de", t_dw2de), ("db2de", t_db2de),
                ("sw1B", t_sw1B), ("sw1C", t_sw1C), ("sb1", t_sb1),
                ("sw2", t_sw2), ("sb2", t_sb2),
                ("mw1", t_mw1), ("mb1", t_mb1),
                ("mw2", t_mw2), ("mb2", t_mb2), ("ones1", onesK_src),
            ]:
                nc.sync.dma_start(out=ttile[:], in_=din[tname][:])

            nc.sync.dma_start(out=Hf[:], in_=h0T[:])
            nc.scalar.copy(out=C[0:D, :], in_=Hf[:])
            nc.sync.dma_start(out=C[96:128, :], in_=hebbT[:])
            nc.sync.dma_start(out=identM[:], in_=identT_in[:])
            nc.sync.dma_start(out=wTblk[:], in_=w0blk[:])
            nc.sync.dma_start(out=idxT[:], in_=idx_in[:])
            ide_b = identM[:].unsqueeze(1).broadcast_to([D_ID, BS, NS])
            nc.scalar.copy(
                out=C[D:96, :].rearrange("p (b n) -> p b n", b=BS),
                in_=ide_b)
            nc.sync.dma_start(out=mshard[:], in_=msh0[:])
            nc.gpsimd.collective_compute(
                "AllGather", ALU.bypass, ins=[mshard.opt()],
                outs=[mfull.opt()], replica_groups=rg)

            # ---------------- time loop ----------------
            for t in range(T):
                nc.sync.dma_start(out=B[D:2 * D, :], in_=injT[t])

                # ---- received: gather + tiny weighted matmuls ----
                for half in range(2):
                    pR = [psRp.tile([64, 512], F32, tag="psR",
                                    name=f"pR{half}_{_b}")
                          for _b in range(BS)]
                    for gi in range(16):
                        g = 16 * half + gi
                        G = gp.tile([128, 8, EL], F32R, tag="G")
                        nc.gpsimd.dma_gather(
                            out_ap=G[:],
                            in_ap=mfull[:],
                            idxs_ap=idxT[:, 64 * g:64 * (g + 1)],
                            num_idxs=1024,
                            num_idxs_reg=1024,
                            elem_size=EL,
                        )
                        for cp in range(8):
                            col = 4 * (8 * gi + cp)
                            for b in range(BS):
                                c_glob = 8 * g + cp
                                nc.tensor.matmul(
                                    pR[b][:, col:col + 4],
                                    G[:, cp, D * b:D * (b + 1)],
                                    wTblk[:, :, b, c_glob],
                                    start=True, stop=True,
                                    skip_group_check=True)
                    # drain received -> B rows 0:64 (b-major cols)
                    for b in range(BS):
                        nc.scalar.copy(
                            out=B[0:D, NS * b + 512 * half:
                                  NS * b + 512 * (half + 1)],
                            in_=pR[b][:])
                    # mod MLP1 for this half's drained chunks
                    if half == 0:
                        modH = hp.tile([128, 2, R], F32R, tag="hid")
                    for b in range(BS):
                        sl = slice(NS * b + 512 * half,
                                   NS * b + 512 * (half + 1))
                        for m in range(2):
                            ps = ps1p.tile([128, 512], F32, tag="mm1")
                            nc.tensor.matmul(
                                ps[:], t_dw1C[:, 128 * m:128 * (m + 1)],
                                C[:, sl], start=True, stop=False)
                            nc.tensor.matmul(
                                ps[:], t_dw1B[:, 128 * m:128 * (m + 1)],
                                B[:, sl], start=False, stop=True)
                            nc.scalar.activation(
                                out=modH[:, m, sl], in_=ps[:], func=AF.Silu,
                                bias=t_db1[:, m:m + 1])

                # ---- mod MLP2 (feature-major outputs) ----
                modHr = modH[:].rearrange("p m (b c g) -> p m b c g",
                                          b=BS, g=4)
                # w^T wrapped: wTblk[32g+k, g, b, c] = sig(w[b, 4c+g, k])
                for g in range(4):
                    for ch in range(2):
                        ps = ps1p.tile([128, 512], F32, tag="mm1")
                        for bi in range(2):
                            b = 2 * ch + bi
                            for m in range(2):
                                nc.tensor.matmul(
                                    ps[0:K, 256 * bi:256 * (bi + 1)],
                                    t_dw2wT[:, m, :],
                                    modHr[:, m, b, :, g],
                                    start=(m == 0), stop=(m == 1),
                                    skip_group_check=True)
                        nc.scalar.activation(
                            out=wTblk[32 * g:32 * (g + 1), g,
                                      2 * ch:2 * (ch + 1), :],
                            in_=ps[0:K, :].rearrange("p (b c) -> p b c", b=2),
                            func=AF.Sigmoid, bias=t_db2wT[:])
                # om = 1 - decay, replicated over 64 partitions via
                # column-replicated stationary
                for q in range(8):
                    sl = slice(512 * q, 512 * (q + 1))
                    ps = ps1p.tile([128, 512], F32, tag="mm1")
                    for m in range(2):
                        nc.tensor.matmul(
                            ps[0:D, :], t_dw2om[:, m, :],
                            modH[:, m, sl],
                            start=(m == 0), stop=(m == 1),
                            skip_group_check=True)
                    nc.scalar.activation(
                        out=omR[:, sl], in_=ps[0:D, :], func=AF.Sigmoid,
                        scale=-1.0, bias=t_db2om[:])
                # ident delta per batch (fm [32, n]); dI slot0 accumulates
                for b in range(BS):
                    slot = 0 if b == 0 else 1
                    for q in range(2):
                        sl = slice(NS * b + 512 * q, NS * b + 512 * (q + 1))
                        ps = ps1p.tile([128, 512], F32, tag="mm1")
                        for m in range(2):
                            nc.tensor.matmul(
                                ps[0:D_ID, :], t_dw2de[:, m, :],
                                modH[:, m, sl],
                                start=(m == 0), stop=(m == 1),
                                skip_group_check=True)
                        nc.scalar.activation(
                            out=dI[:, slot, 512 * q:512 * (q + 1)],
                            in_=ps[0:D_ID, :], func=AF.Identity,
                            bias=(t_db2de[:] if b == 0 else 0.0))
                    if b > 0:
                        nc.vector.tensor_tensor(
                            out=dI[:, 0, :], in0=dI[:, 0, :],
                            in1=dI[:, 1, :], op=ALU.add)

                # ---- ident update + broadcast to C rows 64:96 ----
                nc.vector.scalar_tensor_tensor(
                    out=identM[:], in0=dI[:, 0, :], scalar=1.0 / BS,
                    in1=identM[:], op0=ALU.mult, op1=ALU.add)
                ide_b2 = identM[:].unsqueeze(1).broadcast_to([D_ID, BS, NS])
                nc.scalar.copy(
                    out=C[D:96, :].rearrange("p (b n) -> p b n", b=BS),
                    in_=ide_b2)

                # ---- state MLP1 ----
                stateH = hp.tile([128, 2, R], F32R, tag="hid")
                for q in range(8):
                    sl = slice(512 * q, 512 * (q + 1))
                    for m in range(2):
                        ps = ps1p.tile([128, 512], F32, tag="mm1")
                        nc.tensor.matmul(
                            ps[:], t_sw1B[:, 128 * m:128 * (m + 1)],
                            B[:, sl], start=True, stop=False)
                        nc.tensor.matmul(
                            ps[:], t_sw1C[:, 128 * m:128 * (m + 1)],
                            C[0:96, sl], start=False, stop=True)
                        nc.scalar.activation(
                            out=stateH[:, m, sl], in_=ps[:], func=AF.Silu,
                            bias=t_sb1[:, m:m + 1])

                # ---- state MLP2 feature-major + tanh ----
                for q in range(8):
                    sl = slice(512 * q, 512 * (q + 1))
                    ps = ps1p.tile([128, 512], F32, tag="mm1")
                    for m in range(2):
                        nc.tensor.matmul(
                            ps[0:D, :], t_sw2[:, m, :],
                            stateH[:, m, sl],
                            start=(m == 0), stop=(m == 1),
                            skip_group_check=True)
                    nc.scalar.activation(
                        out=Ttanh[:, sl], in_=ps[0:D, :], func=AF.Tanh,
                        bias=t_sb2[:])

                # ---- h_new = h + om*(tanh - h), feature-major fp32 ----
                nc.gpsimd.tensor_tensor(out=Ttanh[:], in0=Ttanh[:],
                                        in1=Hf[:], op=ALU.subtract)
                nc.gpsimd.tensor_tensor(out=Ttanh[:], in0=Ttanh[:],
                                        in1=omR[:], op=ALU.mult)
                nc.vector.tensor_tensor(out=Hf[:], in0=Hf[:],
                                        in1=Ttanh[:], op=ALU.add)
                nc.scalar.copy(out=C[0:D, :], in_=Hf[:])

                # output word_states for this step (feature-major)
                nc.sync.dma_start(out=out_d[t], in_=Hf[:])

                # ---- msg MLP ----
                msgH = hp.tile([128, 2, R], F32R, tag="hid")
                for q in range(8):
                    sl = slice(512 * q, 512 * (q + 1))
                    for m in range(2):
                        ps = ps1p.tile([128, 512], F32, tag="mm1")
                        nc.tensor.matmul(
                            ps[:], t_mw1[:, 128 * m:128 * (m + 1)],
                            C[0:96, sl], start=True, stop=True)
                        nc.scalar.activation(
                            out=msgH[:, m, sl], in_=ps[:], func=AF.Silu,
                            bias=t_mb1[:, m:m + 1])
                # msg MLP2 rows, n-major interleaved-by-batch output
                for j in range(NS // 128):
                    ps = ps1p.tile([128, 512], F32, tag="mm1")
                    for b in range(BS):
                        rsl = slice(NS * b + 128 * j, NS * b + 128 * (j + 1))
                        nc.tensor.matmul(
                            ps[:, D * b:D * (b + 1)], msgH[:, 0, rsl],
                            t_mw2[:, 0, :], start=True, stop=False,
                            skip_group_check=True)
                        nc.tensor.matmul(
                            ps[:, D * b:D * (b + 1)], msgH[:, 1, rsl],
                            t_mw2[:, 1, :], start=False, stop=False,
                            skip_group_check=True)
                        nc.tensor.matmul(
                            ps[:, D * b:D * (b + 1)], onesK[:],
                            t_mb2[:], start=False, stop=True,
                            skip_group_check=True)
                    nc.scalar.activation(
                        out=msgs[:, j, :], in_=ps[:, 0:EL], func=AF.Tanh)

                # msgs -> DRAM shard -> AllGather
                nc.sync.dma_start(
                    out=mshard[:].rearrange("(j p) d -> p j d", p=128),
                    in_=msgs[:])
                nc.gpsimd.collective_compute(
                    "AllGather", ALU.bypass, ins=[mshard.opt()],
                    outs=[mfull.opt()], replica_groups=rg)

    nc.finalize()
    return nc


def _prep_inputs(inputs):
    """Build the per-core input maps from the full problem inputs."""
    cc = np.asarray(inputs["cc_signals"], dtype=np.float32)
    h0 = np.asarray(inputs["h0"], dtype=np.float32)
    msgs0 = np.asarray(inputs["msgs0"], dtype=np.float32)
    w_conn0 = np.asarray(inputs["w_conn0"], dtype=np.float32)
    hebb = np.asarray(inputs["hebbian"], dtype=np.float32)
    ident = np.asarray(inputs["identity"], dtype=np.float32)
    conn = np.asarray(inputs["conn_indices"]).astype(np.int64)

    def f32(x):
        return np.ascontiguousarray(x, dtype=np.float32)

    def bf16(x):
        return np.ascontiguousarray(
            np.asarray(x, dtype=np.float32).astype(ml_dtypes.bfloat16))

    dw1 = np.asarray(inputs["dw1"], dtype=np.float32)   # [256, 256]
    dw2 = np.asarray(inputs["dw2"], dtype=np.float32)   # [256, 65]
    db2 = np.asarray(inputs["db2"], dtype=np.float32)   # [65]
    sw1 = np.asarray(inputs["sw1"], dtype=np.float32)   # [224, 256]
    sw2 = np.asarray(inputs["sw2"], dtype=np.float32)   # [256, 64]
    mw1 = np.asarray(inputs["mw1"], dtype=np.float32)   # [96, 256]
    mw2 = np.asarray(inputs["mw2"], dtype=np.float32)   # [256, 64]

    # dw1 input order: [hebb(0:32), h(32:96), ide(96:128), rcv, inj]
    # C rows: [h, ide, hebb]; B rows: [rcv, inj]
    shared = {
        "dw1C": f32(np.concatenate([dw1[32:96], dw1[96:128], dw1[0:32]])),
        "dw1B": f32(dw1[128:256]),
        "db1": f32(np.asarray(inputs["db1"]).reshape(2, 128).T),
        "dw2wT": f32(dw2[:, 0:K].reshape(2, 128, K).transpose(1, 0, 2)),
        "db2wT": f32(db2[0:K].reshape(K, 1)),
        "dw2om": f32(np.repeat(dw2[:, K:K + 1], D, axis=1)
                      .reshape(2, 128, D).transpose(1, 0, 2)),
        "db2om": f32(np.full((D, 1), -db2[K])),
        "dw2de": f32(dw2[:, K + 1:].reshape(2, 128, D_ID).transpose(1, 0, 2)),
        "db2de": f32(db2[K + 1:].reshape(D_ID, 1)),
        "sw1B": f32(sw1[0:128]),
        "sw1C": f32(sw1[128:224]),
        "sb1": f32(np.asarray(inputs["sb1"]).reshape(2, 128).T),
        "sw2": f32(sw2.reshape(2, 128, D).transpose(1, 0, 2)),
        "sb2": f32(np.asarray(inputs["sb2"]).reshape(D, 1)),
        "mw1": f32(mw1),
        "mb1": f32(np.asarray(inputs["mb1"]).reshape(2, 128).T),
        "mw2": f32(mw2.reshape(2, 128, D).transpose(1, 0, 2)),
        "mb2": f32(np.asarray(inputs["mb2"]).reshape(1, D)),
        "ones1": f32(np.ones((1, 128))),
    }

    def sigmoid(x):
        return 1.0 / (1.0 + np.exp(-x))

    seg = cc.reshape(BS, T, N // 512, D)  # [b, t, slice, d]
    in_maps = []
    for c in range(NCORES):
        sh = slice(c * NS, (c + 1) * NS)
        m = dict(shared)
        m["h0T"] = f32(h0[:, sh].transpose(2, 0, 1).reshape(D, R))
        m["hebbT"] = f32(hebb[:, sh].transpose(2, 0, 1).reshape(D_ID, R))
        m["identT"] = f32(ident[sh].T)

        injT = np.empty((T, D, BS, NS), dtype=np.float32)
        injT[:, :, :, :512] = seg[:, :, 2 * c].transpose(1, 2, 0)[:, :, :, None]
        injT[:, :, :, 512:] = seg[:, :, 2 * c + 1].transpose(1, 2, 0)[:, :, :, None]
        m["injT"] = f32(injT.reshape(T, D, R))

        # msgs0 interleaved [n, b, d]
        m["msh0"] = f32(msgs0[:, sh].transpose(1, 0, 2).reshape(NS, EL))

        # block-diag wrapped sigmoid(w0): blk[32g+k, g, b, c] = s(w0[b,4c+g,k])
        w0 = sigmoid(w_conn0[:, sh])            # [BS, NS, K]
        wr = w0.reshape(BS, NS // 4, 4, K)      # [b, c, g, k]
        blk = np.zeros((128, 4, BS, NS // 4), dtype=np.float32)
        for g in range(4):
            blk[32 * g:32 * (g + 1), g] = wr[:, :, g, :].transpose(2, 0, 1)
        m["w0blk"] = f32(blk)

        # gather indices: instr g covers targets 32g..32g+32;
        # lin[i] for i = 1024*g + 128*cp + 32*gp + k  -> conn[4*(8g+cp)+gp, k]
        tgt = conn[sh]                          # [1024, 32] global ids
        lin = tgt.reshape(NS // 4, 4, K).reshape(NG, 8, 4, K).reshape(-1)
        wrapped = lin.reshape(2048, 16).T.astype(np.int16)   # [16, 2048]
        m["idx"] = np.ascontiguousarray(np.tile(wrapped, (8, 1)))
        in_maps.append(m)
    return in_maps


def kernel(**inputs) -> np.ndarray:
    key = "prog"
    if key not in _PROGRAM_CACHE:
        _PROGRAM_CACHE[key] = _build_program()
    nc = _PROGRAM_CACHE[key]

    in_maps = _prep_inputs(inputs)
    res = run_bass_kernel_spmd(nc, in_maps, list(range(NCORES)))
    full = np.empty((BS, T, N, D), dtype=np.float32)
    for c in range(NCORES):
        o = np.asarray(res.results[c]["out"]).astype(np.float32)  # [T, D, R]
        o = o.reshape(T, D, BS, NS).transpose(2, 0, 3, 1)
        full[:, :, c * NS:(c + 1) * NS, :] = o
    return full.reshape(BS, T, N // 64, 64 * D)
